# revision 1
# baseline (speedup 1.0000x reference)
"""Trainium2 Bass kernel for MiniKDALayer (chunked delta-rule + gated FFN).

Sequence-parallel over 8 cores (T=8192 -> 1024 rows/core):
  L1 kernel: x^T transpose, fused projections, PoPE, per-chunk (C=64)
     delta-rule quantities via the WY chunked form -> per-chunk affine state
     maps (A^T, B) and chunk-local output pieces (Obase^T, Qeff^T).
  Host: 128-step chunk-state scan over tiny (32,16) states.
  L2 kernel: o assembly, post-RMSNorm, gating, Wout+residual, FFN with
     fp32r matmuls, final residual, straight (t-major) output.
"""
import math

import numpy as np

import concourse.bass as bass
import concourse.bacc as bacc
import concourse.mybir as mybir
import concourse.tile as tile
from concourse.bass_utils import run_bass_kernel_spmd

F32 = mybir.dt.float32
F32R = mybir.dt.float32r
AF = mybir.ActivationFunctionType
OP = mybir.AluOpType

T, D, DK, DKP, DV = 8192, 1024, 16, 32, 16
THETA = 10000.0
EPS = 1.1920929e-07
NCORE = 8
TL = T // NCORE          # 1024 rows per core
C = 64                   # chunk length
NCH = TL // C            # 16 chunks per core
DT = D // 128            # 8 d-tiles
TT = TL // 128           # 8 t-tiles
NW = 128                 # fused projection width (padded, 32-aligned slices)


def r32(ap):
    return ap.bitcast(F32R)


# ---------------------------------------------------------------- L1 builder
def build_l1():
    nc = bacc.Bacc(None, target_bir_lowering=False)
    x = nc.dram_tensor("x", (TL, D), F32, kind="ExternalInput")
    wall = nc.dram_tensor("wall", (D, NW), F32, kind="ExternalInput")
    wa2 = nc.dram_tensor("wa2", (DKP, DKP), F32, kind="ExternalInput")
    trig = nc.dram_tensor("trig", (64, TL), F32, kind="ExternalInput")
    i128 = nc.dram_tensor("i128", (128, 128), F32, kind="ExternalInput")
    maskT = nc.dram_tensor("maskT", (C, C), F32, kind="ExternalInput")
    negmask = nc.dram_tensor("negmask", (C, C), F32, kind="ExternalInput")
    permqk = nc.dram_tensor("permqk", (DKP, 2 * DKP), F32, kind="ExternalInput")

    xT = nc.dram_tensor("xT", (D, TL), F32, kind="ExternalOutput")
    gsig = nc.dram_tensor("gsig", (DV, TL), F32, kind="ExternalOutput")
    obase = nc.dram_tensor("obase", (DV, TL), F32, kind="ExternalOutput")
    qeff = nc.dram_tensor("qeff", (DKP, TL), F32, kind="ExternalOutput")
    amat = nc.dram_tensor("amat", (DKP, NCH * DKP), F32, kind="ExternalOutput")
    bmat = nc.dram_tensor("bmat", (DKP, NCH * DV), F32, kind="ExternalOutput")

    with tile.TileContext(nc) as tc:
        with (
            tc.tile_pool(name="big", bufs=1) as big,
            tc.tile_pool(name="xin", bufs=3) as xin,
            tc.tile_pool(name="ck", bufs=2) as ck,
            tc.tile_pool(name="pst", bufs=3, space="PSUM") as pst,
            tc.tile_pool(name="psp", bufs=2, space="PSUM") as psp,
            tc.tile_pool(name="psc", bufs=3, space="PSUM") as psc,
        ):
            ident = big.tile([128, 128], F32)
            nc.sync.dma_start(out=ident, in_=i128[:, :])
            msk = big.tile([C, C], F32)
            nc.sync.dma_start(out=msk, in_=maskT[:, :])
            nmsk = big.tile([C, C], F32)
            nc.sync.dma_start(out=nmsk, in_=negmask[:, :])
            wallS = big.tile([128, DT, NW], F32)
            nc.sync.dma_start(out=wallS, in_=wall.rearrange("(j p) w -> p j w", p=128))
            wa2S = big.tile([DKP, DKP], F32)
            nc.sync.dma_start(out=wa2S, in_=wa2[:, :])
            trigS = big.tile([64, TL], F32)
            nc.sync.dma_start(out=trigS, in_=trig[:, :])
            permS = big.tile([DKP, 2 * DKP], F32)
            nc.sync.dma_start(out=permS, in_=permqk[:, :])

            # ---- x^T via PE transposes ----
            xTs = [big.tile([128, TL], F32, name=f"xT{j}") for j in range(DT)]
            for i in range(TT):
                for j in range(DT):
                    xt = xin.tile([128, 128], F32, tag="xt")
                    nc.sync.dma_start(
                        out=xt, in_=x[128 * i:128 * i + 128, 128 * j:128 * j + 128])
                    pt = pst.tile([128, 128], F32, tag="ptr")
                    nc.tensor.transpose(pt, xt, ident)
                    nc.scalar.activation(xTs[j][:, 128 * i:128 * i + 128], pt, AF.Copy)
            for j in range(DT):
                nc.sync.dma_start(out=xT[128 * j:128 * j + 128, :], in_=xTs[j])

            # ---- fused projections: psum (97, 512) x2 ----
            # wall cols: 0:16 Wq | 16:32 Wk | 32:64 Wa1 | 64:80 Wv |
            #            96:97 Wbeta | 97:113 Wgate  (rest zero-padded)
            qkmu = big.tile([DKP, TL], F32)   # [-softplus(q); -softplus(k)]
            vT = big.tile([DV, TL], F32)
            a1s = big.tile([DKP, TL], F32)
            gbsg = big.tile([17, TL], F32)    # [sigmoid(beta); sigmoid(gate)]
            spT = big.tile([DKP, TL], F32)
            qksg = big.tile([DKP, TL], F32)
            asg = big.tile([DKP, TL], F32)
            for n in range(2):
                sl = slice(512 * n, 512 * n + 512)
                p = psp.tile([NW, 512], F32, tag="proj")
                for j in range(DT):
                    nc.tensor.matmul(
                        p, wallS[:, j, :], xTs[j][:, sl],
                        start=(j == 0), stop=(j == DT - 1))
                nc.scalar.activation(a1s[:, sl], p[32:64, :], AF.Silu)
                nc.scalar.activation(vT[:, sl], p[64:80, :], AF.Silu)
                # softplus(w) = -ln(sigmoid(-w)); sign folded into trig tables
                nc.scalar.activation(qksg[:, sl], p[0:32, :], AF.Sigmoid, scale=-1.0)
                nc.scalar.activation(gbsg[:, sl], p[96:113, :], AF.Sigmoid)
                pa = psp.tile([DKP, 512], F32, tag="proj")
                nc.tensor.matmul(pa, wa2S, a1s[:, sl], start=True, stop=True)
                nc.scalar.activation(asg[:, sl], pa, AF.Sigmoid)
                nc.scalar.activation(qkmu[:, sl], qksg[:, sl], AF.Ln)
                nc.scalar.activation(spT[:, sl], asg[:, sl], AF.Ln)
            betaT = gbsg[0:1, :]
            nc.sync.dma_start(out=gsig[:, :], in_=gbsg[1:17, :])

            # ---- GL = within-chunk cumsum of log(alpha) = G  (prefix scan) ----
            GN = big.tile([DKP, TL], F32)
            for c in range(NCH):
                cs = slice(C * c, C * c + C)
                nc.vector.tensor_tensor_scan(
                    GN[:, cs], spT[:, cs], spT[:, cs], 0.0, OP.add, OP.bypass)

            # ---- exp factors ----
            eG = big.tile([DKP, TL], F32)
            nc.scalar.activation(eG, GN, AF.Exp)
            eGn = big.tile([DKP, TL], F32)
            nc.scalar.activation(eGn, GN, AF.Exp, scale=-1.0)
            pCall = big.tile([DKP, NCH], F32)
            ebar = big.tile([DKP, TL], F32)
            for c in range(NCH):
                cs = slice(C * c, C * c + C)
                last = slice(C * c + C - 1, C * c + C)
                nc.scalar.activation(pCall[:, c:c + 1], GN[:, last], AF.Exp)
                dg = ck.tile([DKP, C], F32, tag="dg")
                nc.vector.tensor_scalar(dg, GN[:, cs], GN[:, last], None, OP.subtract)
                nc.scalar.activation(ebar[:, cs], dg, AF.Exp, scale=-1.0)

            # ---- PoPE q/k and scaled key variants ----
            q2 = big.tile([DKP, TL], F32)
            k2 = big.tile([DKP, TL], F32)
            # qrep = [qmu;qmu], krep = [kmu;kmu] via permutation matmuls,
            # multiplied by (negated) trig tables on eviction
            for n in range(2):
                sl = slice(512 * n, 512 * n + 512)
                pq = psp.tile([DKP, 512], F32, tag="proj")
                nc.tensor.matmul(pq, permS[:, 0:DKP], qkmu[:, sl],
                                 start=True, stop=True)
                nc.vector.tensor_tensor(q2[:, sl], pq, trigS[0:DKP, sl], OP.mult)
                pk2 = psp.tile([DKP, 512], F32, tag="proj")
                nc.tensor.matmul(pk2, permS[:, DKP:], qkmu[:, sl],
                                 start=True, stop=True)
                nc.vector.tensor_tensor(k2[:, sl], pk2, trigS[DKP:64, sl], OP.mult)
            Qt = big.tile([DKP, TL], F32)
            nc.vector.tensor_tensor(Qt, q2, eG, OP.mult)
            Keta = big.tile([DKP, TL], F32)
            nc.gpsimd.tensor_tensor(Keta, k2, eGn, OP.mult)
            Kkap = big.tile([DKP, TL], F32)
            nc.vector.tensor_tensor(Kkap, k2, eG, OP.mult)
            Kbar = big.tile([DKP, TL], F32)
            nc.gpsimd.tensor_tensor(Kbar, k2, ebar, OP.mult)

            # ---- beta columns: transpose (1,128) pieces -> ball (128, TT) ----
            ball = big.tile([128, TT], F32)
            for i in range(TT):
                pb = pst.tile([128, 1], F32, tag="ptr")
                nc.tensor.transpose(
                    pb, betaT[0:1, 128 * i:128 * i + 128], ident[0:1, 0:1])
                nc.scalar.activation(ball[:, i:i + 1], pb, AF.Copy)

            obS = big.tile([DV, TL], F32)
            qeS = big.tile([DKP, TL], F32)
            amS = big.tile([DKP, NCH * DKP], F32)
            bmS = big.tile([DKP, NCH * DV], F32)

            # ---- per-chunk delta-rule math ----
            for c in range(NCH):
                cs = slice(C * c, C * c + C)
                bcol = ball[C * (c % 2):C * (c % 2) + C, c // 2:c // 2 + 1]

                # AttnT[s,t] (incl mask) ; N = -(strict mask)*M
                pat = psc.tile([C, C], F32, tag="ckm")
                nc.tensor.matmul(pat, Keta[:, cs], Qt[:, cs], start=True, stop=True)
                attnT = ck.tile([C, C], F32, tag="attnT")
                nc.vector.tensor_tensor(attnT, pat, msk, OP.mult)
                pm = psc.tile([C, C], F32, tag="ckm")
                nc.tensor.matmul(pm, Kkap[:, cs], Keta[:, cs], start=True, stop=True)
                nmat = ck.tile([C, C], F32, tag="nmat")
                nc.vector.tensor_tensor(nmat, pm, nmsk, OP.mult)
                npow = ck.tile([C, C], F32, tag="npow")
                nc.vector.tensor_scalar(npow, nmat, bcol, None, OP.mult)

                # R = [b*V | b*Kkap]  (C, 48)
                X = ck.tile([C, DV + DKP], F32, tag="X")
                pv = pst.tile([C, DV], F32, tag="ptr")
                nc.tensor.transpose(pv, vT[:, cs], ident[0:DV, 0:DV])
                nc.vector.tensor_scalar(X[:, 0:DV], pv, bcol, None, OP.mult)
                pk = pst.tile([C, DKP], F32, tag="ptr")
                nc.tensor.transpose(pk, Kkap[:, cs], ident[0:DKP, 0:DKP])
                nc.vector.tensor_scalar(X[:, DV:], pk, bcol, None, OP.mult)

                pkb = pst.tile([C, DKP], F32, tag="ptr")
                nc.tensor.transpose(pkb, Kbar[:, cs], ident[0:DKP, 0:DKP])
                kbar_t = ck.tile([C, DKP], F32, tag="kbar_t")
                nc.scalar.activation(kbar_t, pkb, AF.Copy)

                # X = (I+M)^-1 R via product of (I + N^(2^j)), N = -M
                for j in range(6):
                    pt = pst.tile([C, C], F32, tag="ptr")
                    nc.tensor.transpose(pt, npow, ident[0:C, 0:C])
                    npT = ck.tile([C, C], F32, tag="npT")
                    nc.scalar.activation(npT, pt, AF.Copy)
                    px = psc.tile([C, DV + DKP], F32, tag="ckm")
                    nc.tensor.matmul(px, npT, X, start=True, stop=True)
                    X2 = ck.tile([C, DV + DKP], F32, tag="X")
                    nc.vector.tensor_tensor(X2, X, px, OP.add)
                    X = X2
                    if j < 5:
                        pn2 = psc.tile([C, C], F32, tag="ckm")
                        nc.tensor.matmul(pn2, npT, npow, start=True, stop=True)
                        npow2 = ck.tile([C, C], F32, tag="npow")
                        nc.scalar.activation(npow2, pn2, AF.Copy)
                        npow = npow2

                # Obase^T = Uv^T AttnT ; Qeff^T = Qt^T - W^T AttnT
                pob = psc.tile([DV, C], F32, tag="ckm")
                nc.tensor.matmul(pob, X[:, 0:DV], attnT, start=True, stop=True)
                nc.scalar.activation(obS[:, cs], pob, AF.Copy)
                pqe = psc.tile([DKP, C], F32, tag="ckm")
                nc.tensor.matmul(pqe, X[:, DV:], attnT, start=True, stop=True)
                nc.vector.tensor_tensor(qeS[:, cs], Qt[:, cs], pqe, OP.subtract)

                # A^T = diag(pC) - W^T Kbar ; B = Kbar^T Uv
                pa2 = psc.tile([DKP, DKP], F32, tag="ckm")
                nc.tensor.matmul(pa2, X[:, DV:], kbar_t, start=True, stop=True)
                dgt = ck.tile([DKP, DKP], F32, tag="dgt")
                nc.vector.tensor_scalar(
                    dgt, ident[0:DKP, 0:DKP], pCall[:, c:c + 1], None, OP.mult)
                nc.vector.tensor_tensor(
                    amS[:, DKP * c:DKP * c + DKP], dgt, pa2, OP.subtract)
                pbm = psc.tile([DKP, DV], F32, tag="ckm")
                nc.tensor.matmul(pbm, kbar_t, X[:, 0:DV], start=True, stop=True)
                nc.scalar.activation(bmS[:, DV * c:DV * c + DV], pbm, AF.Copy)

            nc.sync.dma_start(out=obase[:, :], in_=obS)
            nc.sync.dma_start(out=qeff[:, :], in_=qeS)
            nc.sync.dma_start(out=amat[:, :], in_=amS)
            nc.sync.dma_start(out=bmat[:, :], in_=bmS)
    nc.compile()
    return nc


# ---------------------------------------------------------------- L2 builder
def build_l2():
    nc = bacc.Bacc(None, target_bir_lowering=False)
    x = nc.dram_tensor("x", (TL, D), F32, kind="ExternalInput")
    xT = nc.dram_tensor("xT", (D, TL), F32, kind="ExternalInput")
    obase = nc.dram_tensor("obase", (DV, TL), F32, kind="ExternalInput")
    qeff = nc.dram_tensor("qeff", (DKP, TL), F32, kind="ExternalInput")
    gsig = nc.dram_tensor("gsig", (DV, TL), F32, kind="ExternalInput")
    sentry = nc.dram_tensor("sentry", (DKP, NCH * DV), F32, kind="ExternalInput")
    wout = nc.dram_tensor("wout", (DV, D), F32R, kind="ExternalInput")
    pnw = nc.dram_tensor("pnw", (DV, 1), F32, kind="ExternalInput")
    ones = nc.dram_tensor("ones", (128, 128), F32, kind="ExternalInput")
    wg = nc.dram_tensor("wg", (D, D), F32R, kind="ExternalInput")
    wu = nc.dram_tensor("wu", (D, D), F32R, kind="ExternalInput")
    wd = nc.dram_tensor("wd", (D, D), F32R, kind="ExternalInput")
    y = nc.dram_tensor("y", (TL, D), F32, kind="ExternalOutput")

    with tile.TileContext(nc) as tc:
        with (
            tc.tile_pool(name="big", bufs=1) as big,
            tc.tile_pool(name="wpool", bufs=2) as wpool,
            tc.tile_pool(name="work", bufs=3) as work,
            tc.tile_pool(name="psr", bufs=2, space="PSUM") as psr,
            tc.tile_pool(name="psb", bufs=2, space="PSUM") as psb,
            tc.tile_pool(name="psm", bufs=4, space="PSUM") as psm,
        ):
            onesS = big.tile([128, 128], F32)
            nc.sync.dma_start(out=onesS, in_=ones[:, :])
            obS = big.tile([DV, TL], F32)
            nc.sync.dma_start(out=obS, in_=obase[:, :])
            qeS = big.tile([DKP, TL], F32)
            nc.sync.dma_start(out=qeS, in_=qeff[:, :])
            gsS = big.tile([DV, TL], F32)
            nc.sync.dma_start(out=gsS, in_=gsig[:, :])
            seS = big.tile([DKP, NCH * DV], F32)
            nc.sync.dma_start(out=seS, in_=sentry[:, :])
            woutS = big.tile([DV, D], F32R)
            nc.sync.dma_start(out=woutS, in_=wout[:, :])
            pnwS = big.tile([DV, 1], F32)
            nc.sync.dma_start(out=pnwS, in_=pnw[:, :])
            epsS = big.tile([1, 1], F32)
            nc.vector.memset(epsS, EPS)
            x1s = [big.tile([128, TL], F32R, name=f"x1{j}") for j in range(DT)]
            for j in range(DT):
                nc.sync.dma_start(out=x1s[j], in_=r32(xT[128 * j:128 * j + 128, :]))

            # ---- o^T assembly ----
            oT = big.tile([DV, TL], F32)
            for c in range(NCH):
                cs = slice(C * c, C * c + C)
                po = psr.tile([DV, C], F32, tag="red")
                nc.tensor.matmul(po, seS[:, DV * c:DV * c + DV], qeS[:, cs],
                                 start=True, stop=True)
                nc.vector.tensor_tensor(oT[:, cs], obS[:, cs], po, OP.add)

            # ---- post rmsnorm + gate:  og = rms(o)*pnw*gsig ----
            osq = big.tile([DV, TL], F32)
            nc.scalar.activation(osq, oT, AF.Square)
            og = big.tile([DV, TL], F32R)
            for n in range(2):
                sl = slice(512 * n, 512 * n + 512)
                prs = psr.tile([1, 512], F32, tag="red")
                nc.tensor.matmul(prs, onesS[0:DV, 0:1], osq[:, sl],
                                 start=True, stop=True)
                rq = work.tile([1, 512], F32, tag="rq")
                nc.scalar.activation(rq, prs, AF.Sqrt, scale=1.0 / DV, bias=epsS[:, :])
                rr = work.tile([1, 512], F32, tag="rr")
                nc.vector.reciprocal(rr, rq)
                pbv = psr.tile([DV, 512], F32, tag="red")
                nc.tensor.matmul(pbv, onesS[0:1, 0:DV], rr,
                                 start=True, stop=True)
                t1 = work.tile([DV, 512], F32, tag="t1")
                nc.vector.tensor_tensor(t1, oT[:, sl], pbv, OP.mult)
                t2 = work.tile([DV, 512], F32, tag="t2")
                nc.vector.tensor_scalar(t2, t1, pnwS[:, :], None, OP.mult)
                nc.vector.tensor_tensor(og[:, sl], t2, gsS[:, sl], OP.mult)

            # ---- x1^T = x^T + Wout^T og  (in-place into x1s) ----
            for j in range(DT):
                for n in range(2):
                    sl = slice(512 * n, 512 * n + 512)
                    px1 = psm.tile([128, 512], F32, tag="mm")
                    nc.tensor.matmul(px1, woutS[:, 128 * j:128 * j + 128],
                                     og[:, sl], start=True, stop=True)
                    nc.vector.tensor_tensor(x1s[j][:, sl], x1s[j][:, sl], px1, OP.add)

            # ---- ffn rmsnorm -> h (in-place into x1s) ----
            rb = []
            for n in range(2):
                sl = slice(512 * n, 512 * n + 512)
                ph = psr.tile([1, 512], F32, tag="red")
                for j in range(DT):
                    sq = work.tile([128, 512], F32, tag="sq")
                    nc.scalar.activation(sq, x1s[j][:, sl].bitcast(F32), AF.Square)
                    nc.tensor.matmul(ph, onesS[:, 0:1], sq,
                                     start=(j == 0), stop=(j == DT - 1))
                r1q = work.tile([1, 512], F32, tag="r1q")
                nc.scalar.activation(r1q, ph, AF.Sqrt, scale=1.0 / D, bias=epsS[:, :])
                r1 = work.tile([1, 512], F32, tag="r1")
                nc.vector.reciprocal(r1, r1q)
                pbb = psb.tile([128, 512], F32, tag="bcb")
                nc.tensor.matmul(pbb, onesS[0:1, :], r1,
                                 start=True, stop=True)
                rbn = big.tile([128, 512], F32, name=f"rb{n}")
                nc.scalar.activation(rbn, pbb, AF.Copy)
                rb.append(rbn)
            for j in range(DT):
                for n in range(2):
                    sl = slice(512 * n, 512 * n + 512)
                    eng = nc.vector if (j + n) % 2 == 0 else nc.gpsimd
                    eng.tensor_tensor(x1s[j][:, sl], x1s[j][:, sl], rb[n], OP.mult)
            hs = x1s  # now holds h^T

            # ---- gate/up -> z ----
            zs = [big.tile([128, TL], F32R, name=f"z{f}") for f in range(DT)]
            for f in range(DT):
                wgf = wpool.tile([128, DT, 128], F32R, tag="wgf")
                nc.sync.dma_start(
                    out=wgf, in_=wg.rearrange("(j p) f -> p j f", p=128)
                    [:, :, 128 * f:128 * f + 128])
                wuf = wpool.tile([128, DT, 128], F32R, tag="wuf")
                nc.sync.dma_start(
                    out=wuf, in_=wu.rearrange("(j p) f -> p j f", p=128)
                    [:, :, 128 * f:128 * f + 128])
                for n in range(2):
                    sl = slice(512 * n, 512 * n + 512)
                    pg = psm.tile([128, 512], F32, tag="mm")
                    for j in range(DT):
                        nc.tensor.matmul(pg, wgf[:, j, :], hs[j][:, sl],
                                         start=(j == 0), stop=(j == DT - 1))
                    pu = psm.tile([128, 512], F32, tag="mm")
                    for j in range(DT):
                        nc.tensor.matmul(pu, wuf[:, j, :], hs[j][:, sl],
                                         start=(j == 0), stop=(j == DT - 1))
                    gs = work.tile([128, 512], F32, tag="gs")
                    nc.scalar.activation(gs, pg, AF.Silu)
                    nc.vector.tensor_tensor(zs[f][:, sl], pu, gs, OP.mult)

            # ---- down + wout-term + x residual -> y ----
            wds = [big.tile([128, D], F32R, name=f"wd{f}") for f in range(DT)]
            for f in range(DT):
                nc.sync.dma_start(out=wds[f], in_=wd[128 * f:128 * f + 128, :])
            for i in range(TT):
                ts = slice(128 * i, 128 * i + 128)
                for n in range(2):
                    sl = slice(512 * n, 512 * n + 512)
                    pd = psm.tile([128, 512], F32, tag="mm")
                    for f in range(DT):
                        nc.tensor.matmul(pd, zs[f][:, ts], wds[f][:, sl],
                                         start=(f == 0), stop=False)
                    nc.tensor.matmul(pd, og[:, ts], woutS[:, sl],
                                     start=False, stop=True, skip_group_check=True)
                    xin = work.tile([128, 512], F32, tag="xin")
                    nc.sync.dma_start(out=xin, in_=x[ts, sl])
                    yo = work.tile([128, 512], F32, tag="yo")
                    nc.vector.tensor_tensor(yo, pd, xin, OP.add)
                    nc.sync.dma_start(out=y[ts, sl], in_=yo)
    nc.compile()
    return nc


# ---------------------------------------------------------------- host glue
_CACHE = {}


def kernel(**inputs):
    x_seq = np.ascontiguousarray(np.asarray(inputs["x_seq"], np.float32))

    def sigmoid(z):
        return 1.0 / (1.0 + np.exp(-z))

    positions = np.arange(T, dtype=np.float32)
    freqs = THETA ** (np.arange(DK, dtype=np.float32) / DK)
    phi = positions[:, None] * freqs[None, :]
    psi = 2.0 * math.pi * sigmoid(np.asarray(inputs["pope_delta_raw"], np.float32))
    trig_full = -np.concatenate(
        [np.cos(phi).T, np.sin(phi).T, np.cos(phi - psi).T, np.sin(phi - psi).T],
        axis=0).astype(np.float32)  # (64, T); negated: device mu = -softplus
    wall = np.zeros((D, NW), np.float32)
    wall[:, 0:16] = np.asarray(inputs["Wq"], np.float32)
    wall[:, 16:32] = np.asarray(inputs["Wk"], np.float32)
    wall[:, 32:64] = np.asarray(inputs["Wa1"], np.float32)
    wall[:, 64:80] = np.asarray(inputs["Wv"], np.float32)
    wall[:, 96:97] = np.asarray(inputs["Wbeta"], np.float32)
    wall[:, 97:113] = np.asarray(inputs["Wgate"], np.float32)
    i128 = np.eye(128, dtype=np.float32)
    permqk = np.zeros((DKP, 2 * DKP), np.float32)
    for r in range(DKP):
        permqk[r % DK, r] = 1.0          # qrep: out row r <- qmu row r%16
        permqk[DK + r % DK, DKP + r] = 1.0  # krep

    maskT = np.triu(np.ones((C, C), np.float32), 0)
    negmask = np.tril(-np.ones((C, C), np.float32), -1)

    if "l1" not in _CACHE:
        _CACHE["l1"] = build_l1()
    in1 = []
    for m in range(NCORE):
        sl = slice(TL * m, TL * m + TL)
        in1.append({
            "x": np.ascontiguousarray(x_seq[sl]),
            "wall": wall, "wa2": np.ascontiguousarray(inputs["Wa2"]),
            "trig": np.ascontiguousarray(trig_full[:, sl]),
            "i128": i128, "maskT": maskT, "negmask": negmask, "permqk": permqk,
        })
    res1 = run_bass_kernel_spmd(_CACHE["l1"], in1,
                                core_ids=list(range(NCORE))).results

    # host chunk-state scan (128 tiny steps)
    S = np.zeros((DKP, DV), np.float32)
    sentries = []
    for m in range(NCORE):
        se = np.zeros((DKP, NCH * DV), np.float32)
        am, bm = res1[m]["amat"], res1[m]["bmat"]
        for c in range(NCH):
            se[:, DV * c:DV * c + DV] = S
            AT = am[:, DKP * c:DKP * c + DKP]
            B = bm[:, DV * c:DV * c + DV]
            S = AT.T @ S + B
        sentries.append(se)

    ffnw = np.asarray(inputs["ffn_norm_w"], np.float32)[:, None]
    wgm = np.ascontiguousarray(ffnw * np.asarray(inputs["Wffn_gate"], np.float32))
    wum = np.ascontiguousarray(ffnw * np.asarray(inputs["Wffn_up"], np.float32))
    wdm = np.ascontiguousarray(np.asarray(inputs["Wffn_down"], np.float32))
    onesc = np.ones((128, 128), np.float32)

    if "l2" not in _CACHE:
        _CACHE["l2"] = build_l2()
    in2 = []
    for m in range(NCORE):
        sl = slice(TL * m, TL * m + TL)
        in2.append({
            "x": np.ascontiguousarray(x_seq[sl]),
            "xT": res1[m]["xT"], "obase": res1[m]["obase"],
            "qeff": res1[m]["qeff"], "gsig": res1[m]["gsig"],
            "sentry": sentries[m],
            "wout": np.ascontiguousarray(inputs["Wout"]),
            "pnw": np.ascontiguousarray(
                np.asarray(inputs["post_norm_w"], np.float32)[:, None]),
            "ones": onesc, "wg": wgm, "wu": wum, "wd": wdm,
        })
    res2 = run_bass_kernel_spmd(_CACHE["l2"], in2,
                                core_ids=list(range(NCORE))).results
    return np.concatenate([res2[m]["y"] for m in range(NCORE)], axis=0)



# revision 51
# speedup vs baseline: 2.6274x; 2.6274x over previous
"""Trainium2 Bass kernel for MiniKDALayer (chunked delta-rule + gated FFN).

Sequence-parallel over 8 cores (T=8192 -> 1024 rows/core), two launches:
  L1: DMA-transposed bf16 x, fused bf16 projections, PoPE + decay math in a
      partition-packed (128, 256) layout, chunk delta-rule WY math batched
      two chunks per 128-partition tile (block-diagonal masks), Neumann
      inversion with the X-update and N^2 squaring fused into one matmul.
      Outputs per-chunk (pa2, pbm, pC) pieces + obase/qeff/gsig.
  Host: assemble A = diag(pC) - pa2, 128-step chunk-state scan (tiny).
  L2: o assembly from sentries, post-RMSNorm (pnw folded into Wout on
      host), gating, Wout+residual, bf16 FFN with host-packed weights,
      t-major output with f32 x residual.
"""
import math

import numpy as np
import ml_dtypes

import concourse.bass as bass
import concourse.bacc as bacc
import concourse.mybir as mybir
import concourse.tile as tile
from concourse.bass_utils import run_bass_kernel_spmd

F32 = mybir.dt.float32
F32R = mybir.dt.float32r
BF16 = mybir.dt.bfloat16
FP8 = mybir.dt.float8e4
PM = mybir.MatmulPerfMode
AF = mybir.ActivationFunctionType
OP = mybir.AluOpType

T, D, DK, DKP, DV = 8192, 1024, 16, 32, 16
THETA = 10000.0
EPS = 1.1920929e-07
NCORE = 8
TL = T // NCORE          # 1024 rows per core
C = 64                   # chunk length
NCH = TL // C            # 16 chunks per core
NPAIR = NCH // 2         # 8 chunk pairs (128 tokens each)
DT = D // 128            # 8 d-tiles
Q = 4                    # packing quarters: (128, 256) = 4 x (32, 256)
QL = TL // Q             # 256 t-cols per quarter
NBF = np.dtype(ml_dtypes.bfloat16)
NF8 = np.dtype(ml_dtypes.float8_e4m3)
W8SCALE = 16.0


# ---------------------------------------------------------------- L1 builder
def build_l1(stage=99):
    nc = bacc.Bacc(None, target_bir_lowering=False)
    xbf = nc.dram_tensor("xbf", (TL, D), BF16, kind="ExternalInput")
    wallP = nc.dram_tensor("wallP", (128, DT * 128), BF16, kind="ExternalInput")
    wa2 = nc.dram_tensor("wa2", (DKP, DKP), BF16, kind="ExternalInput")
    trigq = nc.dram_tensor("trigq", (128, QL), F32, kind="ExternalInput")
    trigk = nc.dram_tensor("trigk", (128, QL), F32, kind="ExternalInput")
    identb = nc.dram_tensor("identb", (128, 128), BF16, kind="ExternalInput")
    maskp = nc.dram_tensor("maskp", (128, 128), BF16, kind="ExternalInput")
    nmaskp = nc.dram_tensor("nmaskp", (128, 128), BF16, kind="ExternalInput")
    permq = nc.dram_tensor("permq", (128, DKP), BF16, kind="ExternalInput")
    permk = nc.dram_tensor("permk", (128, DKP), BF16, kind="ExternalInput")

    out1 = nc.dram_tensor("out1", (96, TL), BF16, kind="ExternalOutput")
    ambm = nc.dram_tensor("ambm", (DKP, NPAIR * 96), F32, kind="ExternalOutput")
    pcp = nc.dram_tensor("pcp", (128, Q), F32, kind="ExternalOutput")

    with tile.TileContext(nc) as tc:
        with (
            tc.tile_pool(name="big", bufs=1) as big,
            tc.tile_pool(name="prj", bufs=3, space="PSUM") as prj,
            tc.tile_pool(name="pck", bufs=3, space="PSUM") as pck,
            tc.tile_pool(name="pckb", bufs=2, space="PSUM") as pckb,
            tc.tile_pool(name="wk", bufs=3) as wk,
            tc.tile_pool(name="sm", bufs=3) as sm,
        ):
            # ---- constants + x^T (DMA transpose engine, no PE) ----
            identS = big.tile([128, 128], BF16)
            nc.sync.dma_start(out=identS, in_=identb[:, :])
            maskS = big.tile([128, 128], BF16)
            nc.sync.dma_start(out=maskS, in_=maskp[:, :])
            nmaskS = big.tile([128, 128], BF16)
            nc.sync.dma_start(out=nmaskS, in_=nmaskp[:, :])
            wallS = big.tile([128, DT, 128], BF16)
            nc.sync.dma_start(out=wallS, in_=wallP.rearrange("p (j w) -> p j w", w=128))
            wa2S = big.tile([DKP, DKP], BF16)
            nc.sync.dma_start(out=wa2S, in_=wa2[:, :])
            trigqS = big.tile([128, QL], F32)
            nc.sync.dma_start(out=trigqS, in_=trigq[:, :])
            trigkS = big.tile([128, QL], F32)
            nc.sync.dma_start(out=trigkS, in_=trigk[:, :])
            permqS = big.tile([128, DKP], BF16)
            nc.sync.dma_start(out=permqS, in_=permq[:, :])
            permkS = big.tile([128, DKP], BF16)
            nc.sync.dma_start(out=permkS, in_=permk[:, :])
            onesb = big.tile([1, DKP], BF16)
            nc.vector.memset(onesb, 1.0)

            xT = big.tile([128, DT, TL], BF16)
            for j in range(DT):
                nc.sync.dma_start_transpose(
                    out=xT[:, j, :], in_=xbf[:, 128 * j:128 * j + 128])

            if stage == 0:
                nc.sync.dma_start(out=out1[0:96, 0:TL], in_=xT[0:96, 0, :])
            # ---- fused projections (bf16) ----
            # wallP cols: 0:16 Wq | 16:32 Wk | 32:64 Wa1 | 64:65 Wbeta |
            #             65:81 Wgate | 96:112 Wv (rest zero)
            if stage < 1:
                nc.compile_hint = None
            a1s = big.tile([DKP, TL], BF16)
            vT = big.tile([DV, TL], BF16)
            qksgP = big.tile([128, QL], F32)
            outS = big.tile([64, TL], BF16)   # 0:32 qeff | 32:48 obase
            bgT = big.tile([17, TL], BF16)    # row 0 beta | 1:17 gsig
            pp = []
            for n in range(2):
                sl = slice(512 * n, 512 * n + 512)
                p = prj.tile([128, 512], F32, tag="proj")
                for j in range(DT):
                    nc.tensor.matmul(p, wallS[:, j, :], xT[:, j, sl],
                                     start=(j == 0), stop=(j == DT - 1))
                pp.append(p)
            # eviction order groups activation-table sets: silu, then sigmoid
            for n in range(2):
                sl = slice(512 * n, 512 * n + 512)
                nc.scalar.activation(a1s[:, sl], pp[n][32:64, :], AF.Silu)
                nc.scalar.activation(vT[:, sl], pp[n][96:112, :], AF.Silu)
            for n in range(2):
                sl = slice(512 * n, 512 * n + 512)
                # softplus(w) = -ln(sigmoid(-w)); sign folded into trig tables
                for h in range(2):
                    qq = 2 * n + h
                    nc.scalar.activation(
                        qksgP[32 * qq:32 * qq + 32, :],
                        pp[n][0:32, 256 * h:256 * h + 256], AF.Sigmoid, scale=-1.0)
                nc.scalar.activation(bgT[:, sl], pp[n][64:81, :], AF.Sigmoid)

            if stage == 1:
                nc.sync.dma_start(out=out1[0:32, :], in_=a1s)
                nc.sync.dma_start(out=out1[32:48, :], in_=vT)
                nc.sync.dma_start(out=out1[64:81, :], in_=bgT)
            # ---- alpha path: asg = sigmoid(a1s @ wa2), packed ----
            asgP = big.tile([128, QL], F32)
            for n in range(2):
                sl = slice(512 * n, 512 * n + 512)
                pa = prj.tile([128, 512], F32, tag="proj")
                nc.tensor.matmul(pa[0:DKP, :], wa2S, a1s[:, sl],
                                 start=True, stop=True)
                for h in range(2):
                    qq = 2 * n + h
                    nc.scalar.activation(
                        asgP[32 * qq:32 * qq + 32, :],
                        pa[0:DKP, 256 * h:256 * h + 256], AF.Sigmoid)

            # ---- beta broadcast rows (packed (128, 256)) ----
            pbq = prj.tile([128, 512], F32, tag="proj")
            for qq in range(4):
                ps = slice(32 * qq, 32 * qq + 32)
                ts = slice(QL * qq, QL * qq + QL)
                nc.tensor.matmul(pbq[ps, 0:QL], onesb, bgT[0:1, ts],
                                 start=True, stop=True, skip_group_check=True,
                                 tile_position=(0, 32 * qq))
            brepP = big.tile([128, QL], BF16)
            nc.scalar.activation(brepP, pbq[:, 0:QL], AF.Copy)
            brepU = big.tile([DV, TL], BF16)
            for n in range(2):
                sl = slice(512 * n, 512 * n + 512)
                pbu = prj.tile([128, 512], F32, tag="proj")
                nc.tensor.matmul(pbu[0:DV, :], onesb[:, 0:DV], bgT[0:1, sl],
                                 start=True, stop=True)
                nc.scalar.activation(brepU[:, sl], pbu[0:DV, :], AF.Copy)

            # ---- decay pipeline (packed (128, 256)) ----
            spT = big.tile([128, QL], F32)
            nc.scalar.activation(spT, asgP, AF.Ln)
            GN = big.tile([128, QL], F32)
            for k in range(4):
                cs = slice(64 * k, 64 * k + 64)
                nc.vector.tensor_tensor_scan(
                    GN[:, cs], spT[:, cs], spT[:, cs], 0.0, OP.add, OP.bypass)
            eGP = big.tile([128, QL], BF16)
            nc.scalar.activation(eGP, GN, AF.Exp)
            eGnP = big.tile([128, QL], BF16)
            nc.scalar.activation(eGnP, GN, AF.Exp, scale=-1.0)
            dgP = big.tile([128, QL], F32)
            for k in range(4):
                cs = slice(64 * k, 64 * k + 64)
                last = slice(64 * k + 63, 64 * k + 64)
                nc.vector.tensor_scalar(dgP[:, cs], GN[:, cs], GN[:, last],
                                        None, OP.subtract)
            ebarP = big.tile([128, QL], BF16)
            nc.scalar.activation(ebarP, dgP, AF.Exp, scale=-1.0)
            pcS = big.tile([128, Q], F32)
            for k in range(4):
                last = slice(64 * k + 63, 64 * k + 64)
                nc.scalar.activation(pcS[:, k:k + 1], GN[:, last], AF.Exp)
            nc.sync.dma_start(out=pcp[:, :], in_=pcS)

            if stage == 2:
                nc.sync.dma_start(out=out1[0:64, 0:QL], in_=eGP[0:64, :])
                nc.sync.dma_start(out=pcp[:, :], in_=pcS) if False else None
            # ---- PoPE q/k (packed) ----
            qkmuP = big.tile([128, QL], BF16)
            nc.scalar.activation(qkmuP, qksgP, AF.Ln)
            q2k2 = prj.tile([128, 512], F32, tag="proj")
            for qq in range(4):
                ps = slice(32 * qq, 32 * qq + 32)
                nc.tensor.matmul(q2k2[ps, 0:QL], permqS[ps, :], qkmuP[ps, :],
                                 start=True, stop=True, skip_group_check=True,
                                 tile_position=(32 * qq, 32 * qq))
                nc.tensor.matmul(q2k2[ps, QL:512], permkS[ps, :], qkmuP[ps, :],
                                 start=True, stop=True, skip_group_check=True,
                                 tile_position=(32 * qq, 32 * qq))
            q2P = big.tile([128, QL], BF16)
            nc.vector.tensor_tensor(q2P, q2k2[:, 0:QL], trigqS, OP.mult)
            k2P = big.tile([128, QL], BF16)
            nc.vector.tensor_tensor(k2P, q2k2[:, QL:512], trigkS, OP.mult)

            if stage == 3:
                nc.sync.dma_start(out=out1[0:96, 0:QL], in_=q2P[0:96, :])
            # ---- scaled q/k variants (packed, bf16) ----
            QtP = big.tile([128, QL], BF16)
            nc.vector.tensor_tensor(QtP, q2P, eGP, OP.mult)
            KetaP = big.tile([128, QL], BF16)
            nc.gpsimd.tensor_tensor(KetaP, k2P, eGnP, OP.mult)
            KkapP = big.tile([128, QL], BF16)
            nc.vector.tensor_tensor(KkapP, k2P, eGP, OP.mult)
            KbarP = big.tile([128, QL], BF16)
            nc.gpsimd.tensor_tensor(KbarP, k2P, ebarP, OP.mult)

            # ---- Kbeta packed, then stack (80, TL): [bKkap | Kbar | bV] ----
            KbetaP = big.tile([128, QL], BF16)
            nc.vector.tensor_tensor(KbetaP, KkapP, brepP, OP.mult)
            stack = big.tile([80, TL], BF16)
            for n in range(2):
                sl = slice(512 * n, 512 * n + 512)
                nc.vector.tensor_tensor(stack[64:80, sl], vT[:, sl],
                                        brepU[:, sl], OP.mult)
            for qq in range(4):
                ps = slice(32 * qq, 32 * qq + 32)
                ts = slice(QL * qq, QL * qq + QL)
                eng = nc.vector if qq % 2 == 0 else nc.gpsimd
                eng.tensor_copy(stack[0:32, ts], KbetaP[ps, :])
                eng2 = nc.gpsimd if qq % 2 == 0 else nc.vector
                eng2.tensor_copy(stack[32:64, ts], KbarP[ps, :])

            if stage == 4:
                nc.sync.dma_start(out=out1[0:80, :], in_=stack)
            # ---- chunk pairs: delta-rule WY math, 2 chunks per 128 tile ----
            ambmS = big.tile([DKP, NPAIR * 96], F32)
            for p_ in range(NPAIR):
                qq = p_ // 2
                ps = slice(32 * qq, 32 * qq + 32)
                co = slice(128 * (p_ % 2), 128 * (p_ % 2) + 128)
                tl_ = slice(128 * p_, 128 * p_ + 128)

                pat = pck.tile([128, 128], F32, tag="ck")
                nc.tensor.matmul(pat, KetaP[ps, co], QtP[ps, co],
                                 start=True, stop=True,
                                 tile_position=(32 * qq, 0))
                attnT = sm.tile([128, 128], BF16, tag="attnT")
                nc.vector.tensor_tensor(attnT, pat, maskS, OP.mult)

                pm = pck.tile([128, 128], F32, tag="ck")
                nc.tensor.matmul(pm, KbetaP[ps, co], KetaP[ps, co],
                                 start=True, stop=True,
                                 tile_position=(32 * qq, 0))
                W = wk.tile([128, 176], BF16, tag="W")
                nc.vector.tensor_tensor(W[:, 48:176], pm, nmaskS, OP.mult)

                pst = pckb.tile([128, 128], BF16, tag="ckb")
                nc.tensor.transpose(pst[:, 0:80], stack[:, tl_],
                                    identS[0:80, 0:80])
                nc.scalar.activation(W[:, 0:32], pst[:, 0:32], AF.Copy)
                nc.scalar.activation(W[:, 32:48], pst[:, 64:80], AF.Copy)
                kb = sm.tile([128, DKP], BF16, tag="kb")
                nc.vector.tensor_copy(kb, pst[:, 32:64])

                # X = (I+M)^-1 R via product of (I + N^(2^j)); N^2 fused in
                for j in range(6):
                    ptr = pckb.tile([128, 128], BF16, tag="ckb")
                    nc.tensor.transpose(ptr, W[:, 48:176], identS)
                    npT = sm.tile([128, 128], BF16, tag="npT")
                    if j % 2 == 0:
                        nc.scalar.activation(npT, ptr, AF.Copy)
                    else:
                        nc.vector.tensor_copy(npT, ptr)
                    wid = 176 if j < 5 else 48
                    px = pck.tile([128, wid], F32, tag="ck")
                    nc.tensor.matmul(px, npT, W[:, 0:wid], start=True, stop=True)
                    W2 = wk.tile([128, 176], BF16, tag="W")
                    nc.vector.tensor_tensor(W2[:, 0:48], W[:, 0:48],
                                            px[:, 0:48], OP.add)
                    if j < 5:
                        if (j + 1) % 2 == 0:
                            nc.scalar.activation(W2[:, 48:176], px[:, 48:176],
                                                 AF.Copy)
                        else:
                            nc.vector.tensor_copy(W2[:, 48:176], px[:, 48:176])
                    W = W2

                # obase/qeff for the pair
                pobq = pck.tile([128, 128], F32, tag="ck")
                nc.tensor.matmul(pobq[0:48, :], W[:, 0:48], attnT,
                                 start=True, stop=True)
                nc.scalar.activation(outS[32:48, tl_], pobq[32:48, :], AF.Copy)
                nc.vector.tensor_tensor(outS[0:32, tl_], QtP[ps, co],
                                        pobq[0:32, :], OP.subtract)

                # per-chunk A/B pieces: pa2 = X^T kbar, pbm = kbar^T Uv
                pab = pck.tile([DKP, 96], F32, tag="ck")
                for h in range(2):
                    rs = slice(64 * h, 64 * h + 64)
                    nc.tensor.matmul(pab[:, 48 * h:48 * h + 32],
                                     W[rs, 0:32], kb[rs, :],
                                     start=True, stop=True, skip_group_check=True,
                                     tile_position=(64 * h, 0))
                    nc.tensor.matmul(pab[:, 48 * h + 32:48 * h + 48],
                                     kb[rs, :], W[rs, 32:48],
                                     start=True, stop=True, skip_group_check=True,
                                     tile_position=(64 * h, 0))
                nc.scalar.activation(ambmS[:, 96 * p_:96 * p_ + 96], pab, AF.Copy)

            nc.sync.dma_start(out=out1[0:48, :], in_=outS[0:48, :])
            nc.sync.dma_start(out=out1[64:80, :], in_=bgT[1:17, :])
            nc.sync.dma_start(out=ambm[:, :], in_=ambmS)
    nc.compile()
    return nc


# ---------------------------------------------------------------- L2 builder
def build_l2():
    nc = bacc.Bacc(None, target_bir_lowering=False)
    x = nc.dram_tensor("x", (TL, D), F32, kind="ExternalInput")
    xbf = nc.dram_tensor("xbf", (TL, D), BF16, kind="ExternalInput")
    l1o = nc.dram_tensor("l1o", (96, TL), BF16, kind="ExternalInput")
    sentry = nc.dram_tensor("sentry", (DKP, NCH * DV), BF16, kind="ExternalInput")
    wout = nc.dram_tensor("wout", (DV, D), BF16, kind="ExternalInput")
    onesd = nc.dram_tensor("onesd", (128, 128), BF16, kind="ExternalInput")
    identd = nc.dram_tensor("identd", (128, 128), BF16, kind="ExternalInput")
    wg = nc.dram_tensor("wg", (128, DT * D), FP8, kind="ExternalInput")
    wu = nc.dram_tensor("wu", (128, DT * D), FP8, kind="ExternalInput")
    wd = nc.dram_tensor("wd", (128, DT * D), FP8, kind="ExternalInput")
    wout2 = nc.dram_tensor("wout2", (DV, D), BF16, kind="ExternalInput")
    y = nc.dram_tensor("y", (TL, D), F32, kind="ExternalOutput")

    with tile.TileContext(nc) as tc:
        with (
            tc.tile_pool(name="big", bufs=1) as big,
            tc.tile_pool(name="work", bufs=3) as work,
            tc.tile_pool(name="oas", bufs=1, space="PSUM") as oas,
            tc.tile_pool(name="psr", bufs=1, space="PSUM") as psr,
            tc.tile_pool(name="psm", bufs=5, space="PSUM") as psm,
        ):
            l1S = big.tile([48, TL], BF16)
            nc.sync.dma_start(out=l1S, in_=l1o[0:48, :])
            seS = big.tile([DKP, NCH * DV], BF16)
            nc.sync.dma_start(out=seS, in_=sentry[:, :])
            onesS = big.tile([128, 128], BF16)
            nc.sync.dma_start(out=onesS, in_=onesd[:, :])
            identS = big.tile([128, 128], BF16)
            nc.sync.dma_start(out=identS, in_=identd[:, :])
            gsS = big.tile([DV, TL], BF16)
            nc.sync.dma_start(out=gsS, in_=l1o[64:80, :])
            woutS = big.tile([DV, D], BF16)
            nc.sync.dma_start(out=woutS, in_=wout[:, :])
            epsS = big.tile([1, 1], F32)
            nc.vector.memset(epsS, EPS)
            xT = big.tile([128, DT, TL], BF16)
            for j in range(DT):
                nc.sync.dma_start_transpose(
                    out=xT[:, j, :], in_=xbf[:, 128 * j:128 * j + 128])
            # f-outer packed fp8 gate/up weights (DoubleRow: [p, t, i, f*128+c])
            wgS = big.tile([128, 4, 2, D], FP8)
            wuS = big.tile([128, 4, 2, D], FP8)
            for h in range(4):
                fs = slice(2 * h * D, 2 * h * D + 2 * D)
                with tc.tile_wait_until(0.009 + 0.002 * h):
                    nc.sync.dma_start(
                        out=wgS[:, h, :, :],
                        in_=wg[:, fs].rearrange("p (i c) -> p i c", c=D))
                    nc.sync.dma_start(
                        out=wuS[:, h, :, :],
                        in_=wu[:, fs].rearrange("p (i c) -> p i c", c=D))
            wdS = big.tile([128, DT, D], FP8)
            with tc.tile_wait_until(0.020):
                nc.sync.dma_start(out=wdS, in_=wd.rearrange("p (j f) -> p j f", f=D))
            wout2S = big.tile([DV, D], BF16)
            nc.sync.dma_start(out=wout2S, in_=wout2[:, :])

            # ---- o assembly: o = obase + sentry^T qeff ----
            oasm = [oas.tile([DV, 512], F32, name=f"oa{n}") for n in range(2)]
            for c in range(NCH):
                cs = slice(C * c, C * c + C)
                nc.tensor.matmul(oasm[c // 8][:, C * (c % 8):C * (c % 8) + C],
                                 seS[:, DV * c:DV * c + DV],
                                 l1S[0:32, cs],
                                 start=True, stop=True, skip_group_check=True)
            oT = big.tile([DV, TL], F32)
            osq = big.tile([DV, TL], BF16)
            og = big.tile([DV, TL], BF16)
            xT8 = big.tile([128, DT, TL], FP8)
            z8 = big.tile([128, DT, TL], FP8)

            def front(n):
                sl = slice(512 * n, 512 * n + 512)
                nc.vector.tensor_tensor(oT[:, sl], l1S[32:48, sl], oasm[n], OP.add)
                nc.scalar.activation(osq[:, sl], oT[:, sl], AF.Square)
                tg = work.tile([DV, 512], F32, tag="tg")
                nc.vector.tensor_tensor(tg, oT[:, sl], gsS[:, sl], OP.mult)
                prs = psr.tile([128, 512], F32, tag="red")
                nc.tensor.matmul(prs[0:1, :], onesS[0:DV, 0:1], osq[:, sl],
                                 start=True, stop=True)
                rq = work.tile([1, 512], F32, tag="rq")
                nc.scalar.activation(rq, prs[0:1, :], AF.Sqrt, scale=1.0 / DV,
                                     bias=epsS[:, :])
                rr = work.tile([1, 512], F32, tag="rr")
                nc.vector.reciprocal(rr, rq)
                rrb = work.tile([1, 512], BF16, tag="rrb")
                nc.scalar.activation(rrb, rr, AF.Copy)
                pbv = psr.tile([128, 512], F32, tag="red")
                nc.tensor.matmul(pbv[0:DV, :], onesS[0:1, 0:DV], rrb,
                                 start=True, stop=True)
                nc.vector.tensor_tensor(og[:, sl], tg, pbv[0:DV, :], OP.mult)

                for j in range(DT):
                    px1 = psm.tile([128, 512], F32, tag="mm")
                    nc.tensor.matmul(px1, woutS[:, 128 * j:128 * j + 128],
                                     og[:, sl], start=True, stop=False,
                                     skip_group_check=True)
                    nc.tensor.matmul(px1, identS, xT[:, j, sl],
                                     start=False, stop=True,
                                     skip_group_check=True)
                    if j % 2 == 0:
                        nc.scalar.activation(xT[:, j, sl], px1, AF.Copy)
                    else:
                        nc.vector.tensor_copy(xT[:, j, sl], px1)

                ph = psr.tile([128, 512], F32, tag="red")
                for j in range(DT):
                    sq = work.tile([128, 512], BF16, tag="sq")
                    eng = nc.vector if j % 2 == 0 else nc.gpsimd
                    eng.tensor_tensor(sq, xT[:, j, sl], xT[:, j, sl], OP.mult)
                    nc.tensor.matmul(ph[0:1, :], onesS[:, 0:1], sq,
                                     start=(j == 0), stop=(j == DT - 1))
                r1q = work.tile([1, 512], F32, tag="r1q")
                nc.scalar.activation(r1q, ph[0:1, :], AF.Sqrt, scale=1.0 / D,
                                     bias=epsS[:, :])
                r1 = work.tile([1, 512], F32, tag="r1")
                nc.vector.reciprocal(r1, r1q)
                r1b = work.tile([1, 512], BF16, tag="r1b")
                nc.scalar.activation(r1b, r1, AF.Copy)
                pbb = psr.tile([128, 512], F32, tag="red")
                nc.tensor.matmul(pbb[:, :], onesS[0:1, :], r1b,
                                 start=True, stop=True)
                rbn = big.tile([128, 512], BF16, name=f"rb{n}")
                nc.scalar.activation(rbn, pbb, AF.Copy)
                for j in range(DT):
                    eng = nc.vector if j % 2 == 0 else nc.gpsimd
                    eng.tensor_tensor(xT8[:, j, sl], xT[:, j, sl], rbn, OP.mult)

            def gateup(n, f0, f1):
                sl = slice(512 * n, 512 * n + 512)
                for f in range(f0, f1):
                    pg = psm.tile([128, 512], F32, tag="mm")
                    for t in range(4):
                        nc.tensor.matmul(pg, wgS[:, t, :, 128 * f:128 * f + 128],
                                         xT8[:, 2 * t:2 * t + 2, sl],
                                         start=(t == 0), stop=(t == 3),
                                         perf_mode=PM.DoubleRow)
                    pu = psm.tile([128, 512], F32, tag="mm")
                    for t in range(4):
                        nc.tensor.matmul(pu, wuS[:, t, :, 128 * f:128 * f + 128],
                                         xT8[:, 2 * t:2 * t + 2, sl],
                                         start=(t == 0), stop=(t == 3),
                                         perf_mode=PM.DoubleRow)
                    gs = work.tile([128, 512], BF16, tag="gs")
                    nc.scalar.activation(gs, pg, AF.Silu, scale=1.0 / W8SCALE)
                    nc.vector.tensor_tensor(z8[:, f, sl], pu, gs, OP.mult)

            front(0)
            gateup(0, 0, 4)
            front(1)
            gateup(0, 4, DT)
            gateup(1, 0, DT)

            # ---- down (fp8 DoubleRow, x256 scale) + wout-term + residual ----
            for i in range(DT):
                ts = slice(128 * i, 128 * i + 128)
                for n in range(2):
                    sl = slice(512 * n, 512 * n + 512)
                    xin = work.tile([128, 512], F32, tag="xin")
                    with tc.tile_wait_until(0.030 + 0.0012 * (2 * i + n)):
                        nc.scalar.dma_start(out=xin, in_=x[ts, sl])
                    pd = psm.tile([128, 512], F32, tag="mm")
                    for t in range(4):
                        nc.tensor.matmul(pd, z8[:, 2 * t:2 * t + 2, ts],
                                         wdS[:, 2 * t:2 * t + 2, sl],
                                         start=(t == 0), stop=False,
                                         perf_mode=PM.DoubleRow)
                    nc.tensor.matmul(pd, og[:, ts], wout2S[:, sl],
                                     start=False, stop=True, skip_group_check=True)
                    yo = work.tile([128, 512], F32, tag="yo")
                    nc.vector.scalar_tensor_tensor(
                        yo, pd, 1.0 / (W8SCALE * W8SCALE), xin, OP.mult, OP.add)
                    nc.sync.dma_start(out=y[ts, sl], in_=yo)
    nc.compile()
    return nc


# ---------------------------------------------------------------- host glue
_CACHE = {}


def _prep(inputs):
    """Host-side constant preparation (weights packing, trig tables)."""
    def sigmoid(z):
        return 1.0 / (1.0 + np.exp(-z))

    positions = np.arange(T, dtype=np.float32)
    freqs = THETA ** (np.arange(DK, dtype=np.float32) / DK)
    phi = positions[:, None] * freqs[None, :]          # (T, 16)
    psi = 2.0 * math.pi * sigmoid(np.asarray(inputs["pope_delta_raw"], np.float32))
    # negated: device mu = ln(sigmoid(-w)) = -softplus(w)
    trigq_full = -np.concatenate([np.cos(phi).T, np.sin(phi).T], axis=0)  # (32, T)
    trigk_full = -np.concatenate(
        [np.cos(phi - psi).T, np.sin(phi - psi).T], axis=0)

    wall = np.zeros((D, 128), np.float32)
    wall[:, 0:16] = np.asarray(inputs["Wq"], np.float32)
    wall[:, 16:32] = np.asarray(inputs["Wk"], np.float32)
    wall[:, 32:64] = np.asarray(inputs["Wa1"], np.float32)
    wall[:, 64:65] = np.asarray(inputs["Wbeta"], np.float32)
    wall[:, 65:81] = np.asarray(inputs["Wgate"], np.float32)
    wall[:, 96:112] = np.asarray(inputs["Wv"], np.float32)
    # wallP[p, j*128 + w] = wall[128*j + p, w]
    wallP = np.ascontiguousarray(
        wall.reshape(DT, 128, 128).transpose(1, 0, 2).reshape(128, DT * 128)
    ).astype(NBF)

    identb = np.eye(128, dtype=np.float32).astype(NBF)
    onesd = np.ones((128, 128), np.float32).astype(NBF)
    permq1 = np.zeros((DKP, DKP), np.float32)
    permk1 = np.zeros((DKP, DKP), np.float32)
    for f in range(DKP):
        permq1[f % DK, f] = 1.0
        permk1[DK + f % DK, f] = 1.0
    permq = np.tile(permq1, (4, 1))   # (128, 32), replicated per quadrant
    permk = np.tile(permk1, (4, 1))

    tri = np.triu(np.ones((C, C), np.float32), 0)
    ntri = np.tril(-np.ones((C, C), np.float32), -1)
    maskp = np.zeros((128, 128), np.float32)
    nmaskp = np.zeros((128, 128), np.float32)
    for h in range(2):
        maskp[64 * h:64 * h + 64, 64 * h:64 * h + 64] = tri
        nmaskp[64 * h:64 * h + 64, 64 * h:64 * h + 64] = ntri

    ffnw = np.asarray(inputs["ffn_norm_w"], np.float32)[:, None]
    wgm = ffnw * np.asarray(inputs["Wffn_gate"], np.float32)
    wum = ffnw * np.asarray(inputs["Wffn_up"], np.float32)
    wdm = np.asarray(inputs["Wffn_down"], np.float32)

    def packw8(w):  # j-outer fp8: [p, j*D + c] = w[128*j + p, c]
        return np.ascontiguousarray(
            w.reshape(DT, 128, D).transpose(1, 0, 2).reshape(128, DT * D)
        ).astype(NF8)

    def packdr(w):  # DoubleRow fp8: [p, (t, i, f*128+c)] = 16*w[128*(2t+i)+p, ...]
        v = (w * W8SCALE).reshape(4, 2, 128, DT * 128)
        return np.ascontiguousarray(
            v.transpose(2, 0, 1, 3).reshape(128, DT * D)).astype(NF8)

    woutm = (np.asarray(inputs["post_norm_w"], np.float32)[:, None]
             * np.asarray(inputs["Wout"], np.float32)).astype(NBF)

    return {
        "trigq_full": trigq_full, "trigk_full": trigk_full,
        "wallP": wallP, "identb": identb, "onesd": onesd,
        "permq": permq.astype(NBF), "permk": permk.astype(NBF),
        "maskp": maskp.astype(NBF), "nmaskp": nmaskp.astype(NBF),
        "wa2": np.asarray(inputs["Wa2"], np.float32).astype(NBF),
        "wg": packdr(wgm), "wu": packdr(wum),
        "wd": packw8(wdm * W8SCALE),
        "wout": woutm,
        "wout2": (np.asarray(woutm, np.float32) * W8SCALE * W8SCALE).astype(NBF),
    }


def _pack_trig(tr_full, m):
    """(32, T) core-slice -> packed (128, 256): rows 32q hold quarter q."""
    sl = tr_full[:, TL * m:TL * m + TL]                    # (32, 1024)
    return np.ascontiguousarray(
        sl.reshape(32, Q, QL).transpose(1, 0, 2).reshape(128, QL)
    ).astype(np.float32)


def kernel(**inputs):
    x_seq = np.ascontiguousarray(np.asarray(inputs["x_seq"], np.float32))
    xbf = x_seq.astype(NBF)
    cst = _prep(inputs)

    if "l1" not in _CACHE:
        _CACHE["l1"] = build_l1()
    in1 = []
    for m in range(NCORE):
        sl = slice(TL * m, TL * m + TL)
        in1.append({
            "xbf": np.ascontiguousarray(xbf[sl]),
            "wallP": cst["wallP"], "wa2": cst["wa2"],
            "trigq": _pack_trig(cst["trigq_full"], m),
            "trigk": _pack_trig(cst["trigk_full"], m),
            "identb": cst["identb"], "maskp": cst["maskp"],
            "nmaskp": cst["nmaskp"], "permq": cst["permq"],
            "permk": cst["permk"],
        })
    res1 = run_bass_kernel_spmd(_CACHE["l1"], in1,
                                core_ids=list(range(NCORE))).results

    # host chunk-state scan (128 tiny steps on (32, 16) states)
    S = np.zeros((DKP, DV), np.float32)
    sentries = []
    for m in range(NCORE):
        ambm = np.asarray(res1[m]["ambm"], np.float32)
        pcp = np.asarray(res1[m]["pcp"], np.float32)
        se = np.zeros((DKP, NCH * DV), np.float32)
        for c in range(NCH):
            se[:, DV * c:DV * c + DV] = S
            base = 96 * (c // 2) + 48 * (c % 2)
            pa2 = ambm[:, base:base + 32]
            B = ambm[:, base + 32:base + 48]
            pc = pcp[32 * (c // 4):32 * (c // 4) + 32, c % 4]
            AT = np.diag(pc) - pa2
            S = AT.T @ S + B
        sentries.append(se.astype(NBF))

    if "l2" not in _CACHE:
        _CACHE["l2"] = build_l2()
    in2 = []
    for m in range(NCORE):
        sl = slice(TL * m, TL * m + TL)
        in2.append({
            "x": np.ascontiguousarray(x_seq[sl]),
            "xbf": np.ascontiguousarray(xbf[sl]),
            "l1o": res1[m]["out1"], "sentry": sentries[m],
            "wout": cst["wout"], "wout2": cst["wout2"], "onesd": cst["onesd"],
            "identd": cst["identb"],
            "wg": cst["wg"], "wu": cst["wu"], "wd": cst["wd"],
        })
    res2 = run_bass_kernel_spmd(_CACHE["l2"], in2,
                                core_ids=list(range(NCORE))).results
    return np.concatenate([res2[m]["y"] for m in range(NCORE)], axis=0)                ambmS = big.tile([DKP, NPAIR * 96], F32)
                for p_ in range(NPAIR):
                    qq = p_ // 2
                    ps = slice(32 * qq, 32 * qq + 32)
                    co = slice(128 * (p_ % 2), 128 * (p_ % 2) + 128)
                    tl_ = slice(128 * p_, 128 * p_ + 128)
                    pck = (pckA, pckB, pckC)[p_ % 3]
                    tg = ("ckA", "ckB", "ckC")[p_ % 3]

                    patT = pck.tile([128, 256], F32, tag=tg)
                    nc.tensor.matmul(patT, KetaP[ps, co],
                                     qtkb[ps, 0:2, 128 * (p_ % 2):128 * (p_ % 2) + 128],
                                     start=True, stop=True,
                                     tile_position=(32 * qq, 0))
                    attnT = sm.tile([128, 128], BF16, tag="attnT", bufs=2)
                    nc.vector.tensor_tensor(attnT, patT[:, 0:128], maskS, OP.mult)
                    npT = sm.tile([128, 128], BF16, tag="npT", bufs=12)
                    nc.vector.tensor_tensor(npT, patT[:, 128:256], nmaskTS, OP.mult)

                    pm = pck.tile([128, 128], F32, tag=tg)
                    nc.tensor.matmul(pm, KbetaP[ps, co], KetaP[ps, co],
                                     start=True, stop=True,
                                     tile_position=(32 * qq, 0))
                    W = wk.tile([128, 176], BF16, tag="W")
                    nc.vector.tensor_tensor(W[:, 48:176], pm, nmaskS, OP.mult)

                    # stack transpose on the (idle) DMA xbar engine
                    stT = sm.tile([128, 96], BF16, tag="stT", bufs=3)
                    nc.sync.dma_start_transpose(out=stT, in_=stack[:, tl_])

                    # X = (I+M)^-1 R via product of (I + N^(2^j)), truncated
                    # at 5 factors (N^32 term ~1e-6 here). X2 = X + N^(2^j) X
                    # accumulates in PSUM via an identity matmul; npow^2 and
                    # (N^T)^2 = npow^T @ npT ride separate matmuls.
                    NIT = 5
                    for j in range(NIT):
                        Xsrc = stT[:, 0:48] if j == 0 else W[:, 0:48]
                        px = pck.tile([128, 176], F32, tag=tg)
                        nc.tensor.matmul(px[:, 0:48], npT, Xsrc,
                                         start=True, stop=False,
                                         skip_group_check=True)
                        nc.tensor.matmul(px[:, 0:48], identS, Xsrc,
                                         start=False, stop=True,
                                         skip_group_check=True)
                        if j < NIT - 2:
                            nc.tensor.matmul(px[:, 48:176], npT, W[:, 48:176],
                                             start=True, stop=True,
                                             skip_group_check=True)
                        if j < NIT - 1:
                            pnT = pck.tile([128, 128], F32, tag=tg)
                            nc.tensor.matmul(pnT, W[:, 48:176], npT,
                                             start=True, stop=True)
                            npT2 = sm.tile([128, 128], BF16, tag="npT", bufs=12)
                            if j % 2 == 0:
                                nc.scalar.activation(npT2, pnT, AF.Copy)
                            else:
                                nc.vector.tensor_copy(npT2, pnT)
                            npT = npT2
                        W2 = wk.tile([128, 176], BF16, tag="W")
                        wid = 176 if j < NIT - 2 else 48
                        if (j + 1) % 2 == 0:
                            nc.scalar.activation(W2[:, 0:wid], px[:, 0:wid],
                                                 AF.Copy)
                        else:
                            nc.vector.tensor_copy(W2[:, 0:wid], px[:, 0:wid])
                        W = W2

                    # obase/qeff for the pair
                    pobq = pck.tile([128, 128], F32, tag=tg)
                    nc.tensor.matmul(pobq[0:48, :], W[:, 0:48], attnT,
                                     start=True, stop=True)
                    nc.scalar.activation(outS[32:48, tl_], pobq[32:48, :], AF.Copy)
                    nc.vector.tensor_tensor(outS[0:32, tl_], QtP[ps, co],
                                            pobq[0:32, :], OP.subtract)

                    # per-chunk A/B pieces: pa2 = X^T kbar, pbm = kbar^T Uv
                    for h in range(2):
                        rs = slice(64 * h, 64 * h + 64)
                        pab = pck.tile([DKP, 96], F32, tag=tg)
                        nc.tensor.matmul(pab[:, 0:32],
                                         W[rs, 0:32], stT[rs, 64:96],
                                         start=True, stop=True, skip_group_check=True,
                                         tile_position=(64 * h, 0))
                        nc.tensor.matmul(pab[:, 32:48],
                                         stT[rs, 64:96], W[rs, 32:48],
                                         start=True, stop=True, skip_group_check=True,
                                         tile_position=(64 * h, 0))
                        base = 96 * p_ + 48 * h
                        nc.scalar.activation(ambmS[:, base:base + 48],
                                             pab[:, 0:48], AF.Copy)
                    if p_ == 3:
                        nc.sync.dma_start(out=out1[0:48, 0:512],
                                          in_=outS[0:48, 0:512])
                        nc.sync.dma_start(out=ambm[:, 0:384],
                                          in_=ambmS[:, 0:384])


